# revision 30
# baseline (speedup 1.0000x reference)
"""Batched multi-head attention (32 heads, S=2048, D=128, fp32) on 8 Trainium2
NeuronCores. HW-measured ~154 us per launch (prior baseline ~181 us, which was
a serial ACT chain: 128 exps x ~1.19us; the PE itself only needs ~120us).

Sharding: head-parallel - core i computes heads [4i, 4i+4) independently (no
collectives), takes full fp32 inputs, returns the full fp32 output.

Per-core design (4 heads x 2 q-chunks of 1024, 16 sk-tiles each), fp16
matmul operands with fp32 PSUM accumulate (rel err vs fp32 ref ~1.1e-2 vs
the 2e-2 gate):
  - The exp of the 16 score tiles per chunk is split across TWO engines so
    the PE's ~888ns/sk-tile stream rate (2 QK matmuls N=512 + 8 PV matmuls
    N=129, both at issue roofline) sets the pace instead of the ACT:
    sk in {2,5,8,11,13} run on the DVE as a ONE-instruction Schraudolph
    approximation - u = round_i16(score*(SCALE*log2e*1024) + 15304) written
    through an int16 bitcast view of the fp16 pt tile, so bitcast(u) ~=
    exp(score*SCALE)*(1 +- 4%) - the rest on the ACT as exact exp with the
    1/sqrt(D) scale folded into the activation's free affine. The softmax
    denominator (ones column in V, summed by the same PV matmuls) absorbs
    the approximation's common mode; net accuracy cost ~20x under the gate.
    Engine-placement constraints found on HW: a [128,1024] DVE tensor_scalar
    completes in ~1.28us (inside the ~1.34us 2-slot score-psum deadline) but
    its pipe-drain delays the NEXT DVE op to ~2.0us issue-to-issue, so only
    5/16 tiles go to the DVE, spaced >=3 apart, with ACT runs capped at 2.
  - PV runs at LAG 2 (at sk the PE does QK(sk) then PV(sk-2)), so two
    QK+PV pairs (~1.78us) hide under every exp; PV(14) flushes at sk=15 and
    PV(15) at sk=0 of the next chunk. pt pool is 4 deep.
  - Chunk finalize: each PV-accumulator PSUM bank is freed by one fast DVE
    PSUM->SBUF copy (at sk 0,0,2 of the next chunk); recip (strided over the
    ones-column sums) + per-slice tensor_scalar muls + a per-bank store DMA
    run later (sk 3-5) from the SBUF copy. The final chunk pipelines
    copy->recip->mul->store per bank to shrink the tail.
  - Input prep: SWDGE cast-DMA fp32->fp16 into native [s,d] block layout (no
    DMA-transposes - Tile serializes all DMA against an in-flight xbar
    transpose); Q,K transposed 128x128-blockwise on the PE via identity
    matmuls borrowing the PV psum ring, copy-out 3/4 on ACT + 1/4 on DVE.
    Head 0 is quarter-granular (K quarter first, ACT exp-table preloaded at
    t0 via a dummy exp) with only the 3 transposes the first QK needs up
    front; the rest drip into the first chunk's sk loop. V's ones columns
    are set by one strided memset.
"""

import os
import numpy as np

BH, S, D = 32, 2048, 128
N_CORES = 8
HPC = BH // N_CORES  # heads per core
SK = S // 128  # sk tiles per head
SQ = S // 128  # sq subtiles per head
SCALE = 1.0 / float(np.sqrt(D))

# DVE Schraudolph exp: round_i16(x*A + B) bitcast to fp16 ~= exp(x*SCALE).
A_EXP = float(SCALE * np.log2(np.e) * 1024.0)
B_EXP = 15360.0 - 56.0
# sk tiles whose exp runs on the DVE (rest on ACT). The DVE pays a pipe-drain
# bubble of ~(FD-266)/0.96 ns after each op (a [128,1024] tensor_scalar has a
# hard ~2us issue-to-issue floor), so DVE exps are emitted as 4 x 256-col
# pieces (~0.4us each, no bubble) - also releasing the score-psum slot
# piecewise so the QK two sk later never waits.
# 5 DVE tiles, placed so ACT never runs more than 2 exps back-to-back. A
# single [128,1024] tensor_scalar completes in ~1.28us (inside the ~1.34us
# psum-slot deadline); its ~0.8us pipe-drain only taxes the NEXT DVE op,
# which the >=3-sk spacing absorbs.
DVE_SKS = frozenset({2, 5, 8, 11, 13})

_CACHE = {}


def _install_ntff_hook():
    """Provide antenv.axon_hooks (absent in this container) so that
    run_bass_kernel_spmd(trace=True) can capture NTFF profiles."""
    import contextlib, ctypes, sys, types

    if "antenv.axon_hooks" in sys.modules:
        return
    so_path = "/opt/axon/libaxon_pjrt.so"
    hook = None
    try:
        lib = ctypes.CDLL(so_path)
        if hasattr(lib, "axon_start_nrt_profile"):
            lib.axon_start_nrt_profile.argtypes = [
                ctypes.POINTER(ctypes.c_int64),
                ctypes.c_size_t,
            ]
            lib.axon_start_nrt_profile.restype = ctypes.c_int64
            lib.axon_stop_nrt_profile.argtypes = [ctypes.c_char_p]
            lib.axon_stop_nrt_profile.restype = ctypes.c_int64

            @contextlib.contextmanager
            def _h(output_dir, device_ids):
                import jax

                jax.devices()
                if device_ids:
                    ids = (ctypes.c_int64 * len(device_ids))(*device_ids)
                    rc = lib.axon_start_nrt_profile(ids, len(device_ids))
                else:
                    rc = lib.axon_start_nrt_profile(None, 0)
                if rc != 0:
                    raise RuntimeError(f"axon_start_nrt_profile rc={rc}")
                try:
                    yield
                finally:
                    n = lib.axon_stop_nrt_profile(str(output_dir).encode())
                    print(f"ntff profile: {n} file(s) in {output_dir}")

            hook = _h
    except OSError:
        pass
    mod = types.ModuleType("antenv.axon_hooks")
    mod.get_axon_ntff_profile_hook = lambda: hook
    mod.set_axon_ntff_profile_hook = lambda h: None
    sys.modules["antenv.axon_hooks"] = mod


def _split_sync_waits(nc, maxw=1):
    """The walrus codegen in this container rejects instructions carrying more
    than `maxw` sync waits (Tile's scheduler can attach several). Move the
    excess waits onto same-engine nop instructions inserted just before."""
    from concourse import mybir

    n_split = 0
    for f in nc.m.functions:
        for bb in f.blocks:
            out = []
            for inst in bb.instructions:
                si = inst.sync_info
                if si is not None and si.on_wait and len(si.on_wait) > maxw:
                    waits = list(si.on_wait)
                    carriers, keep = waits[:-maxw], waits[-maxw:]
                    si.on_wait = keep
                    inst.sync_info = si
                    for i in range(0, len(carriers), maxw):
                        n_split += 1
                        nop = mybir.InstNoOp(
                            name=f"{inst.name}_wsplit{i}", ins=[], outs=[]
                        )
                        nop.engine = inst.engine
                        nop.sync_info = mybir.SyncInfo(
                            on_wait=carriers[i : i + maxw], on_update=[]
                        )
                        if hasattr(nc, "inst_map"):
                            nc.inst_map[nop.name] = nop
                        out.append(nop)
                out.append(inst)
            bb.instructions[:] = out
    return n_split


def _build():
    import concourse.bass as bass
    from concourse import mybir
    import concourse.tile as tile
    from concourse.masks import make_identity

    fp16 = mybir.dt.float16
    fp32 = mybir.dt.float32
    i16 = mybir.dt.int16
    AF = mybir.ActivationFunctionType
    Alu = mybir.AluOpType

    from concourse.vector_clock import ScopedClock

    class SlimExitTileContext(tile.TileContext):
        def _drain_and_barrier(self, tick_clock, wait_clock):
            nc = self.nc
            drain_inst = nc.sync.drain()
            wait_clock.add_sem_waits(
                drain_inst.ins, ScopedClock({None: tick_clock.global_clock})
            )
            nc.all_engine_barrier()
            assert self.sems is not None
            popped = nc._tile_sem_poison_stack.pop()
            assert popped is self._sem_poison
            nc.clear_and_free_semaphores(list(self.sems.allocated().values()))
            nc.all_engine_barrier(sem_only=True)

    nc = bass.Bass("TRN2", target_bir_lowering=False, debug=False)
    q = nc.dram_tensor("q", [HPC, S, D], fp32, kind="ExternalInput").ap()
    k = nc.dram_tensor("k", [HPC, S, D], fp32, kind="ExternalInput").ap()
    v = nc.dram_tensor("v", [HPC, S, D], fp32, kind="ExternalInput").ap()
    o = nc.dram_tensor("o", [HPC, S, D], fp32, kind="ExternalOutput").ap()

    with SlimExitTileContext(nc) as tc:
        with (
            tc.tile_pool(name="ident", bufs=1) as ident_pool,
            tc.tile_pool(name="native", bufs=4) as native_pool,
            tc.tile_pool(name="qt", bufs=HPC) as qt_pool,
            tc.tile_pool(name="kt", bufs=HPC) as kt_pool,
            tc.tile_pool(name="vsb", bufs=HPC) as v_pool,
            tc.tile_pool(name="pt", bufs=4) as pt_pool,
            tc.tile_pool(name="psum_s", bufs=2, space="PSUM") as psum_s_pool,
            tc.tile_pool(name="psum_o", bufs=4, space="PSUM") as psum_o_pool,
            tc.tile_pool(name="outsb", bufs=3) as out_pool,
            tc.tile_pool(name="pcp", bufs=3) as pcp_pool,
            tc.tile_pool(name="norm", bufs=4) as norm_pool,
        ):
            qts, kts, vsbs, nats = {}, {}, {}, {}

            def cast_inputs(h):
                if h >= HPC:
                    return
                qn = native_pool.tile([128, S], fp16, tag="nat", name=f"qn_{h}")
                nc.gpsimd.dma_start(
                    qn[:].rearrange("p (t d) -> p t d", d=D),
                    q[h].rearrange("(t p) d -> p t d", p=128),
                )
                kn = native_pool.tile([128, S], fp16, tag="nat", name=f"kn_{h}")
                nc.gpsimd.dma_start(
                    kn[:].rearrange("p (t d) -> p t d", d=D),
                    k[h].rearrange("(t p) d -> p t d", p=128),
                )
                nats[h] = (qn, kn)
                vsb = v_pool.tile([128, SK * 129], fp16, tag="vsb", name=f"vsb_{h}")
                vv = vsb[:].rearrange("p (t c) -> p t c", c=129)
                nc.gpsimd.memset(vv[:, :, 128:129], 1.0)
                nc.gpsimd.dma_start(
                    vv[:, :, 0:D], v[h].rearrange("(t p) d -> p t d", p=128)
                )
                vsbs[h] = vsb

            def pe_transpose_part(nat, out, g):
                # borrows the psum_o ring's rotating slots; the PSUM->SBUF
                # copy-out runs mostly on the ACT engine (3 of 4 per chunk)
                # with one on the DVE, balancing the two exp engines
                slot = psum_o_pool.tile(
                    [128, 512], fp32, tag="po", name=f"pox_{nat.name}_{g}"
                )
                for t in range(4):
                    blk = (g * 4 + t) * 128
                    nc.tensor.matmul(
                        slot[:, t * 128 : (t + 1) * 128],
                        nat[:, blk : blk + 128],
                        ident[:],
                        start=(t == 0),
                        stop=True,
                        skip_group_check=True,
                    )
                dst = out[:, g * 512 : (g + 1) * 512]
                if g % 4 == 3:
                    nc.vector.tensor_copy(dst, slot[:])
                else:
                    nc.scalar.copy(dst, slot[:])

            def xpose_part(h, idx):
                """idx 0-3 -> q quarters; 4-7 -> k quarters. One psum
                slot-use each, spread between the QK/exp traffic."""
                if h >= HPC:
                    return
                qn, kn = nats[h]
                if idx == 0:
                    qts[h] = qt_pool.tile([128, S], fp16, tag="qt", name=f"qt_{h}")
                if idx == 4:
                    kts[h] = kt_pool.tile([128, S], fp16, tag="kt", name=f"kt_{h}")
                if idx < 4:
                    pe_transpose_part(qn, qts[h][:], idx)
                else:
                    pe_transpose_part(kn, kts[h][:], idx - 4)

            # ---- head-0 fast start -------------------------------------
            # ACT exp-table preload: one tiny exp at t0 so the ~2.7us
            # ACT_TABLE_LOAD overlaps the input DMAs instead of stalling
            # the first real exp.
            warm = norm_pool.tile([128, 1], fp32, tag="r", name="warm")
            nc.gpsimd.memset(warm[:], 0.0)
            nc.scalar.activation(warm[:], warm[:], AF.Exp, scale=1.0)

            # quarter-granular Q/K cast-DMAs for head 0, K quarter 0 first:
            # the first QK(sk=0) needs kt cols 0:512 and qt cols 0:1024 only.
            qn = native_pool.tile([128, S], fp16, tag="nat", name="qn_0")
            kn = native_pool.tile([128, S], fp16, tag="nat", name="kn_0")
            nats[0] = (qn, kn)
            vsb = v_pool.tile([128, SK * 129], fp16, tag="vsb", name="vsb_0")
            vsbs[0] = vsb
            qv = qn[:].rearrange("p (t d) -> p t d", d=D)
            kv = kn[:].rearrange("p (t d) -> p t d", d=D)
            vv = vsb[:].rearrange("p (t c) -> p t c", c=129)

            def quarter(dst, src, g):
                nc.gpsimd.dma_start(
                    dst[:, 4 * g : 4 * g + 4, :],
                    src[g * 512 : (g + 1) * 512, :].rearrange(
                        "(t p) d -> p t d", p=128
                    ),
                )

            # the three DMAs the first QK needs go first on gpsimd; the
            # identity (also gpsimd - the DVE lacks memset) is built while
            # they transfer, finishing just before the first transpose
            quarter(kv, k[0], 0)
            quarter(qv, q[0], 0)
            quarter(qv, q[0], 1)
            ident = ident_pool.tile([128, 128], fp16)
            make_identity(nc, ident[:])
            nc.gpsimd.memset(vv[:, :, 128:129], 1.0)
            nc.gpsimd.dma_start(
                vv[:, 0:8, 0:D],
                v[0, 0:1024, :].rearrange("(t p) d -> p t d", p=128),
            )
            quarter(kv, k[0], 1)
            quarter(kv, k[0], 2)
            quarter(kv, k[0], 3)
            nc.gpsimd.dma_start(
                vv[:, 8:16, 0:D],
                v[0, 1024:2048, :].rearrange("(t p) d -> p t d", p=128),
            )
            quarter(qv, q[0], 2)
            quarter(qv, q[0], 3)
            # only what QK(sk=0..3, qc=0) needs; the rest drip into the loop
            xpose_part(0, 4)  # kt cols 0:512 (sk 0-3)
            xpose_part(0, 0)  # qt cols 0:512
            xpose_part(0, 1)  # qt cols 512:1024
            # head-0 deferred transposes, emitted at these sk of (h0, qc0)
            H0_XPOSE = {1: 5, 3: 6, 5: 7, 7: 2, 9: 3}

            # ---- compute: flat chunk list, software-pipelined with PV at
            # LAG 2: at sk the PE runs QK(sk) then PV(sk-2), so TWO QK+PV
            # pairs (~1.78us) hide under each exp (~1.2us) and the exp
            # engines (ACT/DVE alternating) never gate the PE. PV(14) is
            # flushed at sk=15 and PV(15) at sk=0 of the next chunk; the
            # previous chunk's PV banks are then freed by three fast
            # PSUM->SBUF copies (sk 0,1) so the new chunk's first PV batch
            # never stalls, and recip+normalize+store run later (sk 3-5)
            # from the SBUF copy (recip on DVE, muls on ACT, store on sync).
            from collections import deque

            pq = deque()  # deferred PV closures, depth 2
            boundary = None  # (copies[3], fins[3]) of the previous chunk
            bank_hooks = []  # final flush: copy_j fires as PV bank j completes
            for h in range(HPC):
                qt, kt, vsb = qts[h], kts[h], vsbs[h]
                for qc in range(2):  # q-chunks of 1024
                    if qc == 0:
                        cast_inputs(h + 1)
                    qbase = qc * 1024
                    po = [
                        psum_o_pool.tile(
                            [128, 3 * 129], fp32, tag="po", name=f"po_h{h}_{qc}_{i}"
                        )
                        for i in range(3)
                    ]

                    def emit_pv(sk, pt, po=po, vsb=vsb):
                        for sq in range(8):
                            dst = po[sq // 3]
                            off = (sq % 3) * 129
                            # start=True clears the ENTIRE psum bank, so only
                            # the first slice written into each bank may carry
                            # it; the other slices' first writes land on
                            # cleared has_written bits and store rather than
                            # accumulate.
                            nc.tensor.matmul(
                                dst[:, off : off + 129],
                                pt[:, sq * 128 : (sq + 1) * 128],
                                vsb[:, sk * 129 : (sk + 1) * 129],
                                start=(sk == 0 and off == 0),
                                stop=(sk == SK - 1),
                                skip_group_check=True,
                            )

                    def make_boundary(po=po, h=h, qbase=qbase):
                        pjs = [None, None, None]

                        def copy(i):
                            # one fast DVE op frees PV bank i for the next
                            # chunk; everything else reads the SBUF copy
                            pj = pcp_pool.tile(
                                [128, 3 * 129], fp32, tag="pc",
                                name=f"pc_{h}_{qbase}_{i}",
                            )
                            nc.vector.tensor_copy(pj[:], po[i][:])
                            pjs[i] = pj

                        def fin(i):
                            nsl = 3 if i < 2 else 2  # bank 2 holds sq 6,7
                            pj = pjs[i]
                            src = pj[:].rearrange("p (t c) -> p t c", c=129)
                            r = norm_pool.tile(
                                [128, nsl], fp32, tag="r", name=f"r_{h}_{qbase}_{i}"
                            )
                            nc.vector.reciprocal(
                                r[:].rearrange("p (t c) -> p t c", c=1),
                                src[:, 0:nsl, 128:129],
                            )
                            ob = out_pool.tile(
                                [128, nsl * D], fp32, tag="ob",
                                name=f"ob_{h}_{qbase}_{i}",
                            )
                            for j in range(nsl):
                                nc.vector.tensor_scalar_mul(
                                    ob[:, j * D : (j + 1) * D],
                                    pj[:, j * 129 : j * 129 + D],
                                    r[:, j : j + 1],
                                )
                            rows = slice(qbase + i * 384, qbase + i * 384 + nsl * 128)
                            nc.sync.dma_start(
                                o[h, rows, :].rearrange("(t p) d -> p t d", p=128),
                                ob[:].rearrange("p (t d) -> p t d", d=D),
                            )

                        return (
                            [lambda i=i: copy(i) for i in range(3)],
                            [lambda i=i: fin(i) for i in range(3)],
                        )

                    for sk in range(SK):
                        ps = psum_s_pool.tile([128, 1024], fp32, tag="ps")
                        for j in range(2):
                            # the two MMs land in the tile's two distinct psum
                            # banks, so each may clear (start) its own bank
                            nc.tensor.matmul(
                                ps[:, j * 512 : (j + 1) * 512],
                                kt[:, sk * 128 : (sk + 1) * 128],
                                qt[:, qbase + j * 512 : qbase + (j + 1) * 512],
                                start=True,
                                stop=True,
                            )
                        pt = pt_pool.tile([128, 1024], fp16)
                        if sk in DVE_SKS:
                            # Schraudolph: round_i16(score*A + B) bits are the
                            # fp16 encoding of ~exp(score*SCALE)
                            nc.vector.tensor_scalar(
                                pt[:].bitcast(i16), ps[:],
                                A_EXP, B_EXP, Alu.mult, Alu.add,
                            )
                        else:
                            nc.scalar.activation(pt[:], ps[:], AF.Exp, scale=SCALE)
                        if h == 0 and qc == 0:
                            if sk in H0_XPOSE:
                                xpose_part(0, H0_XPOSE[sk])
                        elif qc == 1 and sk % 2 == 1:
                            xpose_part(h + 1, (sk - 1) // 2)
                        # lag-2 PV: emit PV(sk-2); at sk=0 also flush the
                        # previous chunk's PV(15)
                        if len(pq) == 2 or (sk == 0 and pq):
                            pq.popleft()()
                        if boundary is not None:
                            copies, fins = boundary
                            if sk == 0:
                                copies[0]()
                                copies[1]()
                            elif sk == 2:
                                copies[2]()
                            elif 3 <= sk <= 5:
                                fins[sk - 3]()
                                if sk == 5:
                                    boundary = None
                        pq.append(lambda sk=sk, pt=pt, f=emit_pv: f(sk, pt))
                        if sk == SK - 1:
                            pq.popleft()()  # PV(14) before the chunk ends
                    boundary = make_boundary()
            pq.popleft()()  # PV(15) of the final chunk
            # per-bank copy->recip->normalize->store pipeline for the tail
            copies, fins = boundary
            for c, f in zip(copies, fins):
                c()
                f()

    _split_sync_waits(nc, maxw=1)
    return nc


def _get_nc():
    if "nc" not in _CACHE:
        _install_ntff_hook()
        _CACHE["nc"] = _build()
    return _CACHE["nc"]


def run_sharded(query, key, value, trace=False, **trace_kwargs):
    """Run the 8-core SPMD kernel; returns (output [BH,S,D] fp32, results obj)."""
    from concourse.bass_utils import run_bass_kernel_spmd

    nc = _get_nc()
    query = np.ascontiguousarray(np.asarray(query, dtype=np.float32))
    key = np.ascontiguousarray(np.asarray(key, dtype=np.float32))
    value = np.ascontiguousarray(np.asarray(value, dtype=np.float32))
    in_maps = [
        {
            "q": query[c * HPC : (c + 1) * HPC],
            "k": key[c * HPC : (c + 1) * HPC],
            "v": value[c * HPC : (c + 1) * HPC],
        }
        for c in range(N_CORES)
    ]
    res = run_bass_kernel_spmd(
        nc, in_maps, list(range(N_CORES)), trace=trace, **trace_kwargs
    )
    out = np.concatenate([r["o"] for r in res.results], axis=0)
    return out, res


def kernel(key, query, value):
    out, _ = run_sharded(query, key, value, trace=False)
    return out


# revision 35
# speedup vs baseline: 1.0040x; 1.0040x over previous
"""Batched multi-head attention (32 heads, S=2048, D=128, fp32) on 8 Trainium2
NeuronCores. HW-measured ~154 us per launch (prior baseline ~181 us, which was
a serial ACT chain: 128 exps x ~1.19us; the PE itself only needs ~120us).

Sharding: head-parallel - core i computes heads [4i, 4i+4) independently (no
collectives), takes full fp32 inputs, returns the full fp32 output.

Per-core design (4 heads x 2 q-chunks of 1024, 16 sk-tiles each), fp16
matmul operands with fp32 PSUM accumulate (rel err vs fp32 ref ~1.1e-2 vs
the 2e-2 gate):
  - The exp of the 16 score tiles per chunk is split across TWO engines so
    the PE's ~888ns/sk-tile stream rate (2 QK matmuls N=512 + 8 PV matmuls
    N=129, both at issue roofline) sets the pace instead of the ACT:
    sk in {2,5,8,11,13} run on the DVE as a ONE-instruction Schraudolph
    approximation - u = round_i16(score*(SCALE*log2e*1024) + 15304) written
    through an int16 bitcast view of the fp16 pt tile, so bitcast(u) ~=
    exp(score*SCALE)*(1 +- 4%) - the rest on the ACT as exact exp with the
    1/sqrt(D) scale folded into the activation's free affine. The softmax
    denominator (ones column in V, summed by the same PV matmuls) absorbs
    the approximation's common mode; net accuracy cost ~20x under the gate.
    Engine-placement constraints found on HW: a [128,1024] DVE tensor_scalar
    completes in ~1.28us (inside the ~1.34us 2-slot score-psum deadline) but
    its pipe-drain delays the NEXT DVE op to ~2.0us issue-to-issue, so only
    5/16 tiles go to the DVE, spaced >=3 apart, with ACT runs capped at 2.
  - PV runs at LAG 2 (at sk the PE does QK(sk) then PV(sk-2)), so two
    QK+PV pairs (~1.78us) hide under every exp; PV(14) flushes at sk=15 and
    PV(15) at sk=0 of the next chunk. pt pool is 4 deep.
  - Chunk finalize: each PV-accumulator PSUM bank is freed by one fast DVE
    PSUM->SBUF copy (at sk 0,0,2 of the next chunk); recip (strided over the
    ones-column sums) + per-slice tensor_scalar muls + a per-bank store DMA
    run later (sk 3-5) from the SBUF copy. The final chunk pipelines
    copy->recip->mul->store per bank to shrink the tail.
  - Input prep: SWDGE cast-DMA fp32->fp16 into native [s,d] block layout (no
    DMA-transposes - Tile serializes all DMA against an in-flight xbar
    transpose); Q,K transposed 128x128-blockwise on the PE via identity
    matmuls borrowing the PV psum ring, copy-out 3/4 on ACT + 1/4 on DVE.
    Head 0 is quarter-granular (K quarter first, ACT exp-table preloaded at
    t0 via a dummy exp) with only the 3 transposes the first QK needs up
    front; the rest drip into the first chunk's sk loop. V's ones columns
    are set by one strided memset.
"""

import os
import numpy as np

BH, S, D = 32, 2048, 128
N_CORES = 8
HPC = BH // N_CORES  # heads per core
SK = S // 128  # sk tiles per head
SQ = S // 128  # sq subtiles per head
SCALE = 1.0 / float(np.sqrt(D))

# DVE Schraudolph exp: round_i16(x*A + B) bitcast to fp16 ~= exp(x*SCALE).
A_EXP = float(SCALE * np.log2(np.e) * 1024.0)
B_EXP = 15360.0 - 56.0
# sk tiles whose exp runs on the DVE (rest on ACT). The DVE pays a pipe-drain
# bubble of ~(FD-266)/0.96 ns after each op (a [128,1024] tensor_scalar has a
# hard ~2us issue-to-issue floor), so DVE exps are emitted as 4 x 256-col
# pieces (~0.4us each, no bubble) - also releasing the score-psum slot
# piecewise so the QK two sk later never waits.
# 5 DVE tiles, placed so ACT never runs more than 2 exps back-to-back. A
# single [128,1024] tensor_scalar completes in ~1.28us (inside the ~1.34us
# psum-slot deadline); its ~0.8us pipe-drain only taxes the NEXT DVE op,
# which the >=3-sk spacing absorbs.
DVE_SKS = frozenset({2, 5, 8, 11, 13})

_CACHE = {}


def _install_ntff_hook():
    """Provide antenv.axon_hooks (absent in this container) so that
    run_bass_kernel_spmd(trace=True) can capture NTFF profiles."""
    import contextlib, ctypes, sys, types

    if "antenv.axon_hooks" in sys.modules:
        return
    so_path = "/opt/axon/libaxon_pjrt.so"
    hook = None
    try:
        lib = ctypes.CDLL(so_path)
        if hasattr(lib, "axon_start_nrt_profile"):
            lib.axon_start_nrt_profile.argtypes = [
                ctypes.POINTER(ctypes.c_int64),
                ctypes.c_size_t,
            ]
            lib.axon_start_nrt_profile.restype = ctypes.c_int64
            lib.axon_stop_nrt_profile.argtypes = [ctypes.c_char_p]
            lib.axon_stop_nrt_profile.restype = ctypes.c_int64

            @contextlib.contextmanager
            def _h(output_dir, device_ids):
                import jax

                jax.devices()
                if device_ids:
                    ids = (ctypes.c_int64 * len(device_ids))(*device_ids)
                    rc = lib.axon_start_nrt_profile(ids, len(device_ids))
                else:
                    rc = lib.axon_start_nrt_profile(None, 0)
                if rc != 0:
                    raise RuntimeError(f"axon_start_nrt_profile rc={rc}")
                try:
                    yield
                finally:
                    n = lib.axon_stop_nrt_profile(str(output_dir).encode())
                    print(f"ntff profile: {n} file(s) in {output_dir}")

            hook = _h
    except OSError:
        pass
    mod = types.ModuleType("antenv.axon_hooks")
    mod.get_axon_ntff_profile_hook = lambda: hook
    mod.set_axon_ntff_profile_hook = lambda h: None
    sys.modules["antenv.axon_hooks"] = mod


def _split_sync_waits(nc, maxw=1):
    """The walrus codegen in this container rejects instructions carrying more
    than `maxw` sync waits (Tile's scheduler can attach several). Move the
    excess waits onto same-engine nop instructions inserted just before."""
    from concourse import mybir

    n_split = 0
    for f in nc.m.functions:
        for bb in f.blocks:
            out = []
            for inst in bb.instructions:
                si = inst.sync_info
                if si is not None and si.on_wait and len(si.on_wait) > maxw:
                    waits = list(si.on_wait)
                    carriers, keep = waits[:-maxw], waits[-maxw:]
                    si.on_wait = keep
                    inst.sync_info = si
                    for i in range(0, len(carriers), maxw):
                        n_split += 1
                        nop = mybir.InstNoOp(
                            name=f"{inst.name}_wsplit{i}", ins=[], outs=[]
                        )
                        nop.engine = inst.engine
                        nop.sync_info = mybir.SyncInfo(
                            on_wait=carriers[i : i + maxw], on_update=[]
                        )
                        if hasattr(nc, "inst_map"):
                            nc.inst_map[nop.name] = nop
                        out.append(nop)
                out.append(inst)
            bb.instructions[:] = out
    return n_split


def _build():
    import concourse.bass as bass
    from concourse import mybir
    import concourse.tile as tile
    from concourse.masks import make_identity

    fp16 = mybir.dt.float16
    fp32 = mybir.dt.float32
    i16 = mybir.dt.int16
    AF = mybir.ActivationFunctionType
    Alu = mybir.AluOpType

    from concourse.vector_clock import ScopedClock

    class SlimExitTileContext(tile.TileContext):
        def _drain_and_barrier(self, tick_clock, wait_clock):
            nc = self.nc
            drain_inst = nc.sync.drain()
            wait_clock.add_sem_waits(
                drain_inst.ins, ScopedClock({None: tick_clock.global_clock})
            )
            nc.all_engine_barrier()
            assert self.sems is not None
            popped = nc._tile_sem_poison_stack.pop()
            assert popped is self._sem_poison
            nc.clear_and_free_semaphores(list(self.sems.allocated().values()))
            nc.all_engine_barrier(sem_only=True)

    nc = bass.Bass("TRN2", target_bir_lowering=False, debug=False)
    q = nc.dram_tensor("q", [HPC, S, D], fp32, kind="ExternalInput").ap()
    k = nc.dram_tensor("k", [HPC, S, D], fp32, kind="ExternalInput").ap()
    v = nc.dram_tensor("v", [HPC, S, D], fp32, kind="ExternalInput").ap()
    o = nc.dram_tensor("o", [HPC, S, D], fp32, kind="ExternalOutput").ap()

    with SlimExitTileContext(nc) as tc:
        with (
            tc.tile_pool(name="ident", bufs=1) as ident_pool,
            tc.tile_pool(name="native", bufs=4) as native_pool,
            tc.tile_pool(name="qt", bufs=HPC) as qt_pool,
            tc.tile_pool(name="kt", bufs=HPC) as kt_pool,
            tc.tile_pool(name="vsb", bufs=HPC) as v_pool,
            tc.tile_pool(name="pt", bufs=4) as pt_pool,
            tc.tile_pool(name="psum_s", bufs=2, space="PSUM") as psum_s_pool,
            tc.tile_pool(name="psum_o", bufs=4, space="PSUM") as psum_o_pool,
            tc.tile_pool(name="outsb", bufs=3) as out_pool,
            tc.tile_pool(name="pcp", bufs=3) as pcp_pool,
            tc.tile_pool(name="norm", bufs=4) as norm_pool,
        ):
            qts, kts, vsbs, nats = {}, {}, {}, {}

            def cast_inputs(h):
                if h >= HPC:
                    return
                qn = native_pool.tile([128, S], fp16, tag="nat", name=f"qn_{h}")
                nc.gpsimd.dma_start(
                    qn[:].rearrange("p (t d) -> p t d", d=D),
                    q[h].rearrange("(t p) d -> p t d", p=128),
                )
                kn = native_pool.tile([128, S], fp16, tag="nat", name=f"kn_{h}")
                nc.gpsimd.dma_start(
                    kn[:].rearrange("p (t d) -> p t d", d=D),
                    k[h].rearrange("(t p) d -> p t d", p=128),
                )
                nats[h] = (qn, kn)
                vsb = v_pool.tile([128, SK * 129], fp16, tag="vsb", name=f"vsb_{h}")
                vv = vsb[:].rearrange("p (t c) -> p t c", c=129)
                nc.gpsimd.memset(vv[:, :, 128:129], 1.0)
                nc.gpsimd.dma_start(
                    vv[:, :, 0:D], v[h].rearrange("(t p) d -> p t d", p=128)
                )
                vsbs[h] = vsb

            def pe_transpose_part(nat, out, g):
                # borrows the psum_o ring's rotating slots; the PSUM->SBUF
                # copy-out runs mostly on the ACT engine (3 of 4 per chunk)
                # with one on the DVE, balancing the two exp engines
                slot = psum_o_pool.tile(
                    [128, 512], fp32, tag="po", name=f"pox_{nat.name}_{g}"
                )
                for t in range(4):
                    blk = (g * 4 + t) * 128
                    nc.tensor.matmul(
                        slot[:, t * 128 : (t + 1) * 128],
                        nat[:, blk : blk + 128],
                        ident[:],
                        start=(t == 0),
                        stop=True,
                        skip_group_check=True,
                    )
                dst = out[:, g * 512 : (g + 1) * 512]
                if g % 4 == 3:
                    nc.vector.tensor_copy(dst, slot[:])
                else:
                    nc.scalar.copy(dst, slot[:])

            def xpose_part(h, idx):
                """idx 0-3 -> q quarters; 4-7 -> k quarters. One psum
                slot-use each, spread between the QK/exp traffic."""
                if h >= HPC:
                    return
                qn, kn = nats[h]
                if idx == 0:
                    qts[h] = qt_pool.tile([128, S], fp16, tag="qt", name=f"qt_{h}")
                if idx == 4:
                    kts[h] = kt_pool.tile([128, S], fp16, tag="kt", name=f"kt_{h}")
                if idx < 4:
                    pe_transpose_part(qn, qts[h][:], idx)
                else:
                    pe_transpose_part(kn, kts[h][:], idx - 4)

            # ---- head-0 fast start -------------------------------------
            # ACT exp-table preload: one tiny exp at t0 so the ~2.7us
            # ACT_TABLE_LOAD overlaps the input DMAs instead of stalling
            # the first real exp.
            warm = norm_pool.tile([128, 1], fp32, tag="r", name="warm")
            nc.gpsimd.memset(warm[:], 0.0)
            nc.scalar.activation(warm[:], warm[:], AF.Exp, scale=1.0)

            # quarter-granular Q/K cast-DMAs for head 0, K quarter 0 first:
            # the first QK(sk=0) needs kt cols 0:512 and qt cols 0:1024 only.
            qn = native_pool.tile([128, S], fp16, tag="nat", name="qn_0")
            kn = native_pool.tile([128, S], fp16, tag="nat", name="kn_0")
            nats[0] = (qn, kn)
            vsb = v_pool.tile([128, SK * 129], fp16, tag="vsb", name="vsb_0")
            vsbs[0] = vsb
            qv = qn[:].rearrange("p (t d) -> p t d", d=D)
            kv = kn[:].rearrange("p (t d) -> p t d", d=D)
            vv = vsb[:].rearrange("p (t c) -> p t c", c=129)

            def quarter(dst, src, g):
                nc.gpsimd.dma_start(
                    dst[:, 4 * g : 4 * g + 4, :],
                    src[g * 512 : (g + 1) * 512, :].rearrange(
                        "(t p) d -> p t d", p=128
                    ),
                )

            # the three DMAs the first QK needs go first on gpsimd; the
            # identity (also gpsimd - the DVE lacks memset) is built while
            # they transfer, finishing just before the first transpose
            quarter(kv, k[0], 0)
            quarter(qv, q[0], 0)
            quarter(qv, q[0], 1)
            ident = ident_pool.tile([128, 128], fp16)
            make_identity(nc, ident[:])
            nc.gpsimd.memset(vv[:, :, 128:129], 1.0)
            nc.gpsimd.dma_start(
                vv[:, 0:8, 0:D],
                v[0, 0:1024, :].rearrange("(t p) d -> p t d", p=128),
            )
            quarter(kv, k[0], 1)
            quarter(kv, k[0], 2)
            quarter(kv, k[0], 3)
            nc.gpsimd.dma_start(
                vv[:, 8:16, 0:D],
                v[0, 1024:2048, :].rearrange("(t p) d -> p t d", p=128),
            )
            quarter(qv, q[0], 2)
            quarter(qv, q[0], 3)
            # only what QK(sk=0..3, qc=0) needs; the rest drip into the loop
            xpose_part(0, 4)  # kt cols 0:512 (sk 0-3)
            xpose_part(0, 0)  # qt cols 0:512
            xpose_part(0, 1)  # qt cols 512:1024
            # head-0 deferred transposes, emitted at these sk of (h0, qc0)
            H0_XPOSE = {1: 5, 3: 6, 5: 7, 7: 2, 9: 3}

            # ---- compute: flat chunk list, software-pipelined with PV at
            # LAG 2: at sk the PE runs QK(sk) then PV(sk-2), so TWO QK+PV
            # pairs (~1.78us) hide under each exp (~1.2us) and the exp
            # engines (ACT/DVE alternating) never gate the PE. PV(14) is
            # flushed at sk=15 and PV(15) at sk=0 of the next chunk; the
            # previous chunk's PV banks are then freed by three fast
            # PSUM->SBUF copies (sk 0,1) so the new chunk's first PV batch
            # never stalls, and recip+normalize+store run later (sk 3-5)
            # from the SBUF copy (recip on DVE, muls on ACT, store on sync).
            from collections import deque

            pq = deque()  # deferred PV closures, depth 2
            boundary = None  # (copies[3], fins[3]) of the previous chunk
            for h in range(HPC):
                qt, kt, vsb = qts[h], kts[h], vsbs[h]
                for qc in range(2):  # q-chunks of 1024
                    if qc == 0:
                        cast_inputs(h + 1)
                    qbase = qc * 1024
                    po = [
                        psum_o_pool.tile(
                            [128, 3 * 129], fp32, tag="po", name=f"po_h{h}_{qc}_{i}"
                        )
                        for i in range(3)
                    ]

                    def emit_pv(sk, pt, po=po, vsb=vsb):
                        for sq in range(8):
                            dst = po[sq // 3]
                            off = (sq % 3) * 129
                            # start=True clears the ENTIRE psum bank, so only
                            # the first slice written into each bank may carry
                            # it; the other slices' first writes land on
                            # cleared has_written bits and store rather than
                            # accumulate.
                            nc.tensor.matmul(
                                dst[:, off : off + 129],
                                pt[:, sq * 128 : (sq + 1) * 128],
                                vsb[:, sk * 129 : (sk + 1) * 129],
                                start=(sk == 0 and off == 0),
                                stop=(sk == SK - 1),
                                skip_group_check=True,
                            )

                    def make_boundary(po=po, h=h, qbase=qbase, final=False):
                        pjs = [None, None, None]
                        # the final chunk's finalize is the kernel tail: run
                        # bank 1 on the ACT so two chains drain in parallel
                        on_act = (lambda i: final and i == 1)

                        def copy(i):
                            # one fast op frees PV bank i for the next
                            # chunk; everything else reads the SBUF copy
                            pj = pcp_pool.tile(
                                [128, 3 * 129], fp32, tag="pc",
                                name=f"pc_{h}_{qbase}_{i}",
                            )
                            if on_act(i):
                                nc.scalar.copy(pj[:], po[i][:])
                            else:
                                nc.vector.tensor_copy(pj[:], po[i][:])
                            pjs[i] = pj

                        def fin(i):
                            nsl = 3 if i < 2 else 2  # bank 2 holds sq 6,7
                            pj = pjs[i]
                            src = pj[:].rearrange("p (t c) -> p t c", c=129)
                            r = norm_pool.tile(
                                [128, nsl], fp32, tag="r", name=f"r_{h}_{qbase}_{i}"
                            )
                            nc.vector.reciprocal(
                                r[:].rearrange("p (t c) -> p t c", c=1),
                                src[:, 0:nsl, 128:129],
                            )
                            ob = out_pool.tile(
                                [128, nsl * D], fp32, tag="ob",
                                name=f"ob_{h}_{qbase}_{i}",
                            )
                            for j in range(nsl):
                                if on_act(i):
                                    nc.scalar.mul(
                                        ob[:, j * D : (j + 1) * D],
                                        pj[:, j * 129 : j * 129 + D],
                                        r[:, j : j + 1],
                                    )
                                else:
                                    nc.vector.tensor_scalar_mul(
                                        ob[:, j * D : (j + 1) * D],
                                        pj[:, j * 129 : j * 129 + D],
                                        r[:, j : j + 1],
                                    )
                            rows = slice(qbase + i * 384, qbase + i * 384 + nsl * 128)
                            nc.sync.dma_start(
                                o[h, rows, :].rearrange("(t p) d -> p t d", p=128),
                                ob[:].rearrange("p (t d) -> p t d", d=D),
                            )

                        return (
                            [lambda i=i: copy(i) for i in range(3)],
                            [lambda i=i: fin(i) for i in range(3)],
                        )

                    for sk in range(SK):
                        ps = psum_s_pool.tile([128, 1024], fp32, tag="ps")
                        for j in range(2):
                            # the two MMs land in the tile's two distinct psum
                            # banks, so each may clear (start) its own bank
                            nc.tensor.matmul(
                                ps[:, j * 512 : (j + 1) * 512],
                                kt[:, sk * 128 : (sk + 1) * 128],
                                qt[:, qbase + j * 512 : qbase + (j + 1) * 512],
                                start=True,
                                stop=True,
                            )
                        pt = pt_pool.tile([128, 1024], fp16)
                        if sk in DVE_SKS:
                            # Schraudolph: round_i16(score*A + B) bits are the
                            # fp16 encoding of ~exp(score*SCALE)
                            nc.vector.tensor_scalar(
                                pt[:].bitcast(i16), ps[:],
                                A_EXP, B_EXP, Alu.mult, Alu.add,
                            )
                        else:
                            nc.scalar.activation(pt[:], ps[:], AF.Exp, scale=SCALE)
                        if h == 0 and qc == 0:
                            if sk in H0_XPOSE:
                                xpose_part(0, H0_XPOSE[sk])
                        elif qc == 1 and sk % 2 == 1:
                            xpose_part(h + 1, (sk - 1) // 2)
                        # lag-2 PV: emit PV(sk-2); at sk=0 also flush the
                        # previous chunk's PV(15)
                        if len(pq) == 2 or (sk == 0 and pq):
                            pq.popleft()()
                        if boundary is not None:
                            copies, fins = boundary
                            if sk == 0:
                                copies[0]()
                                copies[1]()
                            elif sk == 2:
                                copies[2]()
                            elif 3 <= sk <= 5:
                                fins[sk - 3]()
                                if sk == 5:
                                    boundary = None
                        pq.append(lambda sk=sk, pt=pt, f=emit_pv: f(sk, pt))
                        if sk == SK - 1:
                            pq.popleft()()  # PV(14) before the chunk ends
                    boundary = make_boundary(final=(h == HPC - 1 and qc == 1))
            pq.popleft()()  # PV(15) of the final chunk
            # per-bank copy->recip->normalize->store pipeline for the tail;
            # bank 1 drains on the ACT in parallel with banks 0/2 on the DVE
            copies, fins = boundary
            copies[1]()
            copies[0]()
            fins[0]()
            fins[1]()
            copies[2]()
            fins[2]()

    _split_sync_waits(nc, maxw=1)
    return nc


def _get_nc():
    if "nc" not in _CACHE:
        _install_ntff_hook()
        _CACHE["nc"] = _build()
    return _CACHE["nc"]


def run_sharded(query, key, value, trace=False, **trace_kwargs):
    """Run the 8-core SPMD kernel; returns (output [BH,S,D] fp32, results obj)."""
    from concourse.bass_utils import run_bass_kernel_spmd

    nc = _get_nc()
    query = np.ascontiguousarray(np.asarray(query, dtype=np.float32))
    key = np.ascontiguousarray(np.asarray(key, dtype=np.float32))
    value = np.ascontiguousarray(np.asarray(value, dtype=np.float32))
    in_maps = [
        {
            "q": query[c * HPC : (c + 1) * HPC],
            "k": key[c * HPC : (c + 1) * HPC],
            "v": value[c * HPC : (c + 1) * HPC],
        }
        for c in range(N_CORES)
    ]
    res = run_bass_kernel_spmd(
        nc, in_maps, list(range(N_CORES)), trace=trace, **trace_kwargs
    )
    out = np.concatenate([r["o"] for r in res.results], axis=0)
    return out, res


def kernel(key, query, value):
    out, _ = run_sharded(query, key, value, trace=False)
    return out
